# revision 15
# baseline (speedup 1.0000x reference)
"""Bass/Trainium2 kernel for nn_DProdQ_2448131359012 (vq_codebook).

For x [N=131072, 512], codebook [4, 256, 128], rotate [512, 512], computes
hard_codes [N, 4] int32 (per-subspace argmax of -L2 distance over K=256
rotated-product-quantizer codes) and the scalar quantization loss.

Sharding: data-parallel over N across 8 NeuronCores; codebook/rotate
replicated; per-partition loss partial sums combined on the host.

Per-core pipeline (supertile = 512 rows, 4 subtiles of 128):
  - x ingested pre-transposed via DMA xbar transpose (2x 64-partition
    transfers per 128-d chunk) -> xT [128d, 512n].
  - xrT[m] (= splits.T) via float32r matmuls with a host-side Dekker split
    of rotate (hi/lo), exact to ~2^-18 relative.
  - t = x.c - c2/2 in PSUM: rank-1 (ones x -c2/2) opens the accumulation
    group, per-subspace fp32 matmuls (lhsT = splitsT chunk, rhs = cbT)
    accumulate on top.
  - tmax (3D reduce_max), punnorm = exp(2t - 2tmax) on ACT with fused
    denominator accum, onehot = (punnorm == 1) bf16, k* = sum(onehot*iota)
    via fused tensor_tensor_reduce per subspace.
  - probs normalized bf16, PE-transposed, softT = cb.T @ probsT,
    sh = soft @ cbT for the soft.hard term.
  - Loss terms accumulate per-partition via chained TTRs:
    sum splits^2, soft^2, splits.soft, soft.hard, c2[k*], tmax.
"""

import numpy as np

M, K, D = 4, 256, 512
DM = D // M  # 128
N_FULL = 131072
N_CORES = 8

XC_F32R = False  # False: exact fp32 xc matmul; True: fast fp32r (truncated)

import os
STAGE = int(os.environ.get("KSTAGE", "9"))  # pipeline truncation for bisection


def build_kernel(nc, rows):
    """Emit the tile program for one core processing `rows` rows."""
    from contextlib import ExitStack
    import concourse.tile as tile
    from concourse import mybir

    f32 = mybir.dt.float32
    f32r = mybir.dt.float32r
    bf16 = mybir.dt.bfloat16
    i32 = mybir.dt.int32
    Alu = mybir.AluOpType
    Act = mybir.ActivationFunctionType
    AX = mybir.AxisListType

    n_st = rows // 512
    xc_dt = f32r if XC_F32R else f32

    xh_d = nc.dram_tensor("x_hi", [rows, D], bf16, kind="ExternalInput").ap()
    xl_d = nc.dram_tensor("x_lo", [rows, D], bf16, kind="ExternalInput").ap()
    roth_d = nc.dram_tensor("rot_hi", [128, 4, 512], f32r, kind="ExternalInput").ap()
    rotl_d = nc.dram_tensor("rot_lo", [128, 4, 512], f32r, kind="ExternalInput").ap()
    cbt_d = nc.dram_tensor("cbT", [128, 4, 256], xc_dt, kind="ExternalInput").ap()
    cbs_d = nc.dram_tensor("cb_sb", [128, 4, 2, 128], bf16, kind="ExternalInput").ap()
    cbtb_d = nc.dram_tensor("cbT_bf", [128, 4, 256], bf16, kind="ExternalInput").ap()
    nc2_d = nc.dram_tensor("negc2h", [1, 1024], f32r, kind="ExternalInput").ap()
    ones_d = nc.dram_tensor("ones1", [1, 128], f32r, kind="ExternalInput").ap()
    c2b_d = nc.dram_tensor("c2b", [128, 4, 256], bf16, kind="ExternalInput").ap()
    iota_d = nc.dram_tensor("iota_b", [128, 4, 256], bf16, kind="ExternalInput").ap()
    ident_d = nc.dram_tensor("ident", [128, 128], bf16, kind="ExternalInput").ap()
    hc_d = nc.dram_tensor("hard_codes", [rows, M], i32, kind="ExternalOutput").ap()
    acc_d = nc.dram_tensor("partials", [128, 12], f32, kind="ExternalOutput").ap()

    # accumulator columns
    A_TMAX = 0   # 0..3 sum tmax per subspace
    A_SP2 = 4    # sum splits^2
    A_SO2 = 5    # sum soft^2
    A_SPSO = 6   # sum splits.soft
    A_C2S = 7    # sum c2[k*]
    A_SOHA = 8   # sum soft.hard

    ctx = ExitStack()
    with ctx:
        tc = ctx.enter_context(tile.TileContext(nc))
        singles = ctx.enter_context(tc.tile_pool(name="singles", bufs=1))
        xt_pool = ctx.enter_context(tc.tile_pool(name="xt", bufs=2))
        xr_ps_pool = ctx.enter_context(tc.tile_pool(name="xr_ps", bufs=1, space="PSUM"))
        xr_sb_pool = ctx.enter_context(tc.tile_pool(name="xr_sb", bufs=2))
        t_ps_pool = ctx.enter_context(tc.tile_pool(name="t_ps", bufs=1, space="PSUM"))
        pt_ps_pool = ctx.enter_context(tc.tile_pool(name="pt_ps", bufs=1, space="PSUM"))
        so_ps_pool = ctx.enter_context(tc.tile_pool(name="so_ps", bufs=1, space="PSUM"))
        sh_ps_pool = ctx.enter_context(tc.tile_pool(name="sh_ps", bufs=1, space="PSUM"))
        work = ctx.enter_context(tc.tile_pool(name="work", bufs=2))
        small = ctx.enter_context(tc.tile_pool(name="small", bufs=4))
        outp = ctx.enter_context(tc.tile_pool(name="outp", bufs=3))

        roth = singles.tile([128, 4, 512], f32r, tag="roth")
        rotl = singles.tile([128, 4, 512], f32r, tag="rotl")
        cbt = singles.tile([128, 4, 256], xc_dt, tag="cbt")
        cbs = singles.tile([128, 4, 2, 128], bf16, tag="cbs")
        cbtb = singles.tile([128, 4, 256], bf16, tag="cbtb")
        nc2 = singles.tile([1, 1024], f32r, tag="nc2")
        ones1 = singles.tile([1, 128], f32r, tag="ones1")
        c2b = singles.tile([128, 4, 256], bf16, tag="c2b")
        iota = singles.tile([128, 4, 256], bf16, tag="iota")
        ident = singles.tile([128, 128], bf16, tag="ident")
        acc = singles.tile([128, 12], f32, tag="acc")
        # TTR mandatory elementwise outputs (scratch, values unused)
        scr_a = singles.tile([128, 2, 512], bf16, tag="scr_a")
        scr_b = singles.tile([128, 4, 256], bf16, tag="scr_b")
        scr_c = singles.tile([128, 4, 128], bf16, tag="scr_c")

        for sb, dr in [(roth, roth_d), (rotl, rotl_d), (cbt, cbt_d), (cbs, cbs_d),
                       (cbtb, cbtb_d), (nc2, nc2_d), (ones1, ones_d), (c2b, c2b_d),
                       (iota, iota_d), (ident, ident_d)]:
            nc.sync.dma_start(out=sb[:], in_=dr)
        nc.vector.memset(acc[:], 0.0)

        for st in range(n_st):
            r0 = st * 512
            # ---- x load, transposed (bf16 hi/lo halves via xbar) ----
            xth = xt_pool.tile([128, 4, 512], bf16, tag="xth")
            xtl = xt_pool.tile([128, 4, 512], bf16, tag="xtl")
            for c in range(4):
                nc.sync.dma_start(
                    out=xth[:, c, :],
                    in_=xh_d[r0:r0 + 512, c * 128:(c + 1) * 128],
                    transpose=True)
                nc.sync.dma_start(
                    out=xtl[:, c, :],
                    in_=xl_d[r0:r0 + 512, c * 128:(c + 1) * 128],
                    transpose=True)
            xt = xt_pool.tile([128, 4, 512], f32r, tag="xt")
            if os.environ.get("KGPS", "1") == "0" or STAGE == 0:
                nc.vector.tensor_tensor(out=xt[:], in0=xth[:], in1=xtl[:],
                                        op=Alu.add)
            else:
                nc.gpsimd.tensor_tensor(out=xt[:], in0=xth[:], in1=xtl[:],
                                        op=Alu.add)
            if STAGE < 1:
                nc.vector.tensor_copy(out=acc[:, 0:4],
                                      in_=xt[:, 0:1, 0:4])
                continue
            # ---- xrT = splitsT, Dekker hi+lo, two psum banks at a time ----
            xrsb = xr_sb_pool.tile([128, 4, 512], xc_dt, tag="xrsb")
            for half in range(2):
                xr_ps = xr_ps_pool.tile([128, 2, 512], f32, tag="xr_ps")
                for mi in range(2):
                    m = half * 2 + mi
                    for c in range(4):
                        nc.tensor.matmul(
                            xr_ps[:, mi, :],
                            lhsT=roth[:, c, m * 128:(m + 1) * 128],
                            rhs=xt[:, c, :],
                            start=(c == 0), stop=False)
                    for c in range(4):
                        nc.tensor.matmul(
                            xr_ps[:, mi, :],
                            lhsT=rotl[:, c, m * 128:(m + 1) * 128],
                            rhs=xt[:, c, :],
                            start=False, stop=(c == 3))
                    if mi == 0:
                        nc.scalar.copy(out=xrsb[:, m, :], in_=xr_ps[:, mi, :])
                    else:
                        nc.vector.tensor_copy(out=xrsb[:, m, :], in_=xr_ps[:, mi, :])

            # sum splits^2
            tsp2 = small.tile([128, 1], f32, tag="tsp2")
            nc.scalar.activation(out=scr_a[:], in_=xrsb[:, 0:2, :],
                                 func=Act.Square, accum_out=tsp2[:])
            nc.vector.tensor_tensor(out=acc[:, A_SP2:A_SP2 + 1],
                                    in0=acc[:, A_SP2:A_SP2 + 1],
                                    in1=tsp2[:], op=Alu.add)
            nc.scalar.activation(out=scr_a[:], in_=xrsb[:, 2:4, :],
                                 func=Act.Square, accum_out=tsp2[:])
            nc.vector.tensor_tensor(out=acc[:, A_SP2:A_SP2 + 1],
                                    in0=acc[:, A_SP2:A_SP2 + 1],
                                    in1=tsp2[:], op=Alu.add)

            for sub in range(4):
                if STAGE < 2:
                    break
                ns = slice(sub * 128, (sub + 1) * 128)
                # ---- t = x.c - c2/2 ----
                t_ps = t_ps_pool.tile([128, 4, 256], f32, tag="t_ps")
                nc.tensor.matmul(t_ps[:, 0:2, :], lhsT=ones1[:],
                                 rhs=nc2[:, 0:512], start=True, stop=False)
                nc.tensor.matmul(t_ps[:, 2:4, :], lhsT=ones1[:],
                                 rhs=nc2[:, 512:1024], start=True, stop=False)
                for m in range(M):
                    nc.tensor.matmul(
                        t_ps[:, m, :], lhsT=xrsb[:, m, ns], rhs=cbt[:, m, :],
                        start=False, stop=(m % 2 == 1))
                # ---- softmax pieces ----
                tmax = small.tile([128, 4], f32, tag="tmax")
                nc.vector.reduce_max(out=tmax[:], in_=t_ps[:], axis=AX.X)
                neg2t = small.tile([128, 4], f32, tag="neg2t")
                nc.vector.tensor_scalar_mul(neg2t[:], tmax[:], -2.0)
                if STAGE < 3:
                    continue
                punnorm = work.tile([128, 4, 256], bf16, tag="punnorm")
                denom = small.tile([128, 4], f32, tag="denom")
                for m in range(M):
                    nc.scalar.activation(
                        out=punnorm[:, m, :], in_=t_ps[:, m, :], func=Act.Exp,
                        bias=neg2t[:, m:m + 1], scale=2.0,
                        accum_out=denom[:, m:m + 1])
                onehot = work.tile([128, 4, 256], bf16, tag="onehot")
                nc.vector.tensor_scalar(
                    out=onehot[:], in0=punnorm[:],
                    scalar1=1.0, scalar2=None, op0=Alu.is_equal)
                kstar = small.tile([128, 4], f32, tag="kstar")
                nc.vector.tensor_tensor(out=scr_b[:], in0=onehot[:],
                                        in1=iota[:], op=Alu.mult)
                nc.vector.reduce_sum(out=kstar[:], in_=scr_b[:], axis=AX.X)
                hc = outp.tile([128, 4], i32, tag="hc")
                nc.vector.tensor_copy(out=hc[:], in_=kstar[:])
                nc.sync.dma_start(
                    out=hc_d[r0 + sub * 128: r0 + (sub + 1) * 128, :], in_=hc[:])
                if STAGE < 4:
                    continue
                # ---- probs ----
                rden = small.tile([128, 4], f32, tag="rden")
                nc.vector.reciprocal(out=rden[:], in_=denom[:])
                probs = work.tile([128, 4, 256], bf16, tag="probs")
                for m in range(M):
                    nc.vector.tensor_scalar_mul(
                        probs[:, m, :], punnorm[:, m, :], rden[:, m:m + 1])
                # ---- transpose probs on PE ----
                pt_ps = pt_ps_pool.tile([128, 8, 128], bf16, tag="pt_ps")
                for m in range(M):
                    for kc in range(2):
                        nc.tensor.transpose(
                            pt_ps[:, m * 2 + kc, :],
                            probs[:, m, kc * 128:(kc + 1) * 128],
                            ident[:])
                pt = work.tile([128, 8, 128], bf16, tag="pt")
                nc.vector.tensor_copy(out=pt[:], in_=pt_ps[:])
                if STAGE < 5:
                    continue
                # ---- softT ----
                so_ps = so_ps_pool.tile([128, 4, 128], f32, tag="so_ps")
                for m in range(M):
                    for kc in range(2):
                        nc.tensor.matmul(
                            so_ps[:, m, :], lhsT=cbs[:, m, kc, :],
                            rhs=pt[:, m * 2 + kc, :],
                            start=(kc == 0), stop=(kc == 1))
                softbf = work.tile([128, 4, 128], bf16, tag="softbf")
                nc.scalar.copy(out=softbf[:], in_=so_ps[:])
                # ---- sh = soft @ cbT ----
                sh_ps = sh_ps_pool.tile([128, 4, 256], f32, tag="sh_ps")
                for m in range(M):
                    nc.tensor.matmul(
                        sh_ps[:, m, :], lhsT=softbf[:, m, :], rhs=cbtb[:, m, :],
                        start=(m % 2 == 0), stop=(m % 2 == 1))
                if STAGE < 6:
                    continue
                # ---- loss accumulations (all chained into acc) ----
                tred = small.tile([128, 1], f32, tag="tred")
                nc.vector.tensor_tensor(out=scr_c[:], in0=xrsb[:, :, ns],
                                        in1=so_ps[:], op=Alu.mult)
                nc.vector.reduce_sum(out=tred[:], in_=scr_c[:], axis=AX.XY)
                nc.vector.tensor_tensor(out=acc[:, A_SPSO:A_SPSO + 1],
                                        in0=acc[:, A_SPSO:A_SPSO + 1],
                                        in1=tred[:], op=Alu.add)
                tso2 = small.tile([128, 1], f32, tag="tso2")
                nc.scalar.activation(out=scr_c[:], in_=softbf[:],
                                     func=Act.Square, accum_out=tso2[:])
                nc.vector.tensor_tensor(out=acc[:, A_SO2:A_SO2 + 1],
                                        in0=acc[:, A_SO2:A_SO2 + 1],
                                        in1=tso2[:], op=Alu.add)
                nc.vector.tensor_tensor(out=scr_b[:], in0=onehot[:],
                                        in1=c2b[:], op=Alu.mult)
                nc.vector.reduce_sum(out=tred[:], in_=scr_b[:], axis=AX.XY)
                nc.vector.tensor_tensor(out=acc[:, A_C2S:A_C2S + 1],
                                        in0=acc[:, A_C2S:A_C2S + 1],
                                        in1=tred[:], op=Alu.add)
                nc.vector.tensor_tensor(out=scr_b[:], in0=onehot[:],
                                        in1=sh_ps[:], op=Alu.mult)
                nc.vector.reduce_sum(out=tred[:], in_=scr_b[:], axis=AX.XY)
                nc.vector.tensor_tensor(out=acc[:, A_SOHA:A_SOHA + 1],
                                        in0=acc[:, A_SOHA:A_SOHA + 1],
                                        in1=tred[:], op=Alu.add)
                nc.vector.tensor_tensor(
                    out=acc[:, A_TMAX:A_TMAX + 4],
                    in0=acc[:, A_TMAX:A_TMAX + 4], in1=tmax[:], op=Alu.add)

        nc.sync.dma_start(out=acc_d, in_=acc[:])
    return nc


def split_x(x):
    """Dekker split of x into bf16 hi + bf16 lo (hi + lo == x exactly in f32
    up to the ~2^-17 tail lost in lo's own bf16 rounding)."""
    import ml_dtypes

    bf16 = ml_dtypes.bfloat16
    x_hi = x.astype(bf16)
    x_lo = (x - x_hi.astype(np.float32)).astype(bf16)
    return np.ascontiguousarray(x_hi), np.ascontiguousarray(x_lo)


def host_prep(codebook, rotate):
    """Static device tensors derived from codebook/rotate (host side)."""
    import ml_dtypes

    bf16 = ml_dtypes.bfloat16
    f32 = np.float32
    rot_hi = rotate.astype(bf16).astype(f32)
    rot_lo = (rotate - rot_hi).astype(f32)
    roth = np.ascontiguousarray(rot_hi.reshape(4, 128, 512).transpose(1, 0, 2))
    rotl = np.ascontiguousarray(rot_lo.reshape(4, 128, 512).transpose(1, 0, 2))
    cbT = np.ascontiguousarray(codebook.transpose(2, 0, 1))  # [128, 4, 256]
    cb_sb = np.ascontiguousarray(
        codebook.reshape(4, 2, 128, 128).transpose(2, 0, 1, 3)).astype(bf16)
    cbT_bf = cbT.astype(bf16)
    c2 = (codebook.astype(np.float64) ** 2).sum(-1).astype(f32)  # [4, 256]
    negc2h = np.ascontiguousarray((-0.5 * c2).reshape(1, 1024))
    c2b = np.ascontiguousarray(
        np.broadcast_to(c2.astype(bf16)[None], (128, 4, 256)))
    iota_b = np.ascontiguousarray(np.broadcast_to(
        np.arange(256, dtype=f32).astype(bf16)[None, None], (128, 4, 256)))
    ones1 = np.ones((1, 128), f32)
    ident = np.eye(128, dtype=f32).astype(bf16)
    return {"rot_hi": roth, "rot_lo": rotl, "cbT": cbT, "cb_sb": cb_sb,
            "cbT_bf": cbT_bf, "negc2h": negc2h, "ones1": ones1, "c2b": c2b,
            "iota_b": iota_b, "ident": ident}


def combine(hc_list, acc_list, rotate, n_total):
    """Host-side unshard: assemble hard_codes and the scalar loss."""
    hard_codes = np.concatenate(hc_list, axis=0).astype(np.int32)
    a = np.stack([p.astype(np.float64) for p in acc_list]).sum(axis=(0, 1))
    sum_tmax = a[0:4].sum()
    s_sp2, s_so2, s_spso, s_c2s, s_soha = a[4], a[5], a[6], a[7], a[8]
    denom_el = float(n_total) * DM
    soft_d = (s_sp2 - 2.0 * s_spso + s_so2) / denom_el
    hard_d = (s_sp2 - 2.0 * sum_tmax) / denom_el
    joint = (s_so2 - 2.0 * s_soha + s_c2s) / denom_el
    r = rotate.astype(np.float32)
    rrt = r @ r.T
    reg = ((rrt - np.eye(D, dtype=np.float32)) ** 2).mean(dtype=np.float64)
    loss = 0.1 * soft_d + hard_d + 0.1 * joint + 0.01 * reg
    return hard_codes, np.float32(loss)


def kernel(x, codebook, rotate):
    import sys
    for p in ("/opt/trn_rl_repo", "/root/.axon_site/_ro/trn_rl_repo"):
        if p not in sys.path:
            sys.path.insert(0, p)
    from concourse import bacc, bass_utils

    x = np.ascontiguousarray(np.asarray(x, dtype=np.float32))
    codebook = np.ascontiguousarray(np.asarray(codebook, dtype=np.float32))
    rotate = np.ascontiguousarray(np.asarray(rotate, dtype=np.float32))
    n_total = x.shape[0]
    rows = n_total // N_CORES

    nc = bacc.Bacc("TRN2", target_bir_lowering=False, debug=False)
    build_kernel(nc, rows)
    nc.compile()

    statics = host_prep(codebook, rotate)
    x_hi, x_lo = split_x(x)
    in_maps = []
    for c in range(N_CORES):
        m = {"x_hi": x_hi[c * rows:(c + 1) * rows],
             "x_lo": x_lo[c * rows:(c + 1) * rows]}
        m.update(statics)
        in_maps.append(m)

    res = bass_utils.run_bass_kernel_spmd(nc, in_maps, core_ids=list(range(N_CORES)))
    hc_list = [r["hard_codes"] for r in res.results]
    acc_list = [r["partials"] for r in res.results]
    return combine(hc_list, acc_list, rotate, n_total)
